# revision 1
# baseline (speedup 1.0000x reference)
"""Trainium2 Bass kernel for the masked contrastive (MIL/NCE-style) loss.

Computes, for instance embeddings x[b,n,:], bag embeddings y[k,:]:
    logits[b,n,k] = cos(x[b,n], y[k]) / T
    loss = -mean_{mask}( logits[b,n,b] - logsumexp_{k!=b} logits[b,n,k] )

Strategy: data-parallel over bags across 8 NeuronCores (32 bags = 8192
instance rows per core). Every core receives the full bag embedding,
rolled by its bag offset so that each core's own-bag diagonal lands at a
fixed, compile-time-known column. Each core emits per-partition partial
sums of the masked per-instance terms and of the mask; the host does the
final (tiny) reduction and division.

Per-core math: raw[r,k] = x[r]·(y[k]/||y[k]||); s[r] = (1/T)/||x[r]||;
logits = s*raw.  Since |logits| <= 1/T = 2, exp never overflows and no
max-subtraction is needed; the k==b exclusion is handled by subtracting
exp(diag) from the full row-sum of exp.  Row norms come from the Gram
diagonal computed on the TensorE (sharing stationary weights with the
logits matmul), and rsqrt is exp(-0.5*ln(ss)) so the ScalarE only ever
needs the natural_log_exp_and_others table set (one table load total).
"""

import os
import sys

import numpy as np

for _p in ("/opt/trn_rl_repo",):
    if os.path.isdir(_p) and _p not in sys.path:
        sys.path.append(_p)

B, N, D = 256, 256, 768
NCORES = 8
BPC = B // NCORES          # bags per core = 32
RPC = BPC * N              # instance rows per core = 8192
P = 128                    # partitions
NT = RPC // P              # row tiles per core = 64
DC = D // P                # contraction chunks = 6
K = B                      # logits columns = 256
GROUP = 4                  # tiles per rsqrt batch (bounded by PSUM banks)
EPS2 = 1e-16               # eps^2 for the norm clamp (eps = 1e-8)
LN2 = 0.6931471805599453   # ln(2) == ln(1/T) for T=0.5

_CACHE = {}


def _patch_act_tables():
    """Prefer the natural_log_exp_and_others ACT table set so Exp, Ln,
    Square and Copy all resolve to ONE resident table (the default
    first-match order picks exp_and_others for Exp and natural_log for
    Ln, reloading tables ~38x per kernel)."""
    import concourse.bacc as bacc
    import concourse.hw_specs as hw_specs

    if getattr(hw_specs, "_ct_patched", False):
        return
    orig = hw_specs.get_activation_tables

    def patched(module_arch):
        # IMPORTANT: set order (and therefore act_func_set_id indices) must
        # stay identical to act_info.json — walrus/NRT resolve the id by
        # file index.  So instead of reordering we hide Exp/Ln from every
        # other set, forcing the chooser onto the combined set.
        import concourse.mybir as mybir

        AF = mybir.ActivationFunctionType
        tabs = orig(module_arch)
        pref = "natural_log_exp_and_others"
        if pref not in tabs:
            return tabs
        return {
            name: (fns if name == pref else fns - {AF.Exp, AF.Ln})
            for name, fns in tabs.items()
        }

    hw_specs.get_activation_tables = patched
    hw_specs._ct_patched = True
    if getattr(bacc, "get_activation_tables", None) is orig:
        bacc.get_activation_tables = patched


def _build(repeat=1, cp_act=320, merge=2, xbufs=4, itbufs=3, scrbufs=3,
           group=2, tpbufs=2, grbufs=2, lgbufs=4, trans="pe",
           use_fp8=True, pair_lg=False, fp8_x=False, debug_out=False,
           compile_=True):
    """Build + compile the single-core SPMD program. cp_act: how many of
    the 768 transpose-copy columns go to ScalarE (rest to VectorE);
    merge: instance-row tiles loaded per (casting) DMA."""
    from contextlib import ExitStack

    import concourse.bacc as bacc
    import concourse.mybir as mybir
    import concourse.tile as tile
    from concourse.masks import make_identity

    _patch_act_tables()

    dt = mybir.dt
    AF = mybir.ActivationFunctionType
    ALU = mybir.AluOpType
    f32 = dt.float32
    bf16 = dt.bfloat16
    fp8 = dt.float8e4
    import math
    SC = 16.0  # fp8 pre-scale on normalized bag rows (folded into s)
    s_bias = LN2 - (math.log(SC) if use_fp8 else 0.0)

    nc = bacc.Bacc("TRN2", target_bir_lowering=False, debug=False,
                   num_devices=NCORES)
    inst = nc.dram_tensor("inst", [RPC, D], f32, kind="ExternalInput").ap()
    bag = nc.dram_tensor("bag", [K, D], f32, kind="ExternalInput").ap()
    maskT = nc.dram_tensor("maskT", [P, NT], dt.int32,
                           kind="ExternalInput").ap()
    out = nc.dram_tensor("out", [P, 2], f32, kind="ExternalOutput").ap()
    dbg = (nc.dram_tensor("dbg", [P, 5 * NT], f32, kind="ExternalOutput").ap()
           if debug_out else None)

    with tile.TileContext(nc) as tc, ExitStack() as ctx:
        consts = ctx.enter_context(tc.tile_pool(name="consts", bufs=1))
        xpool = ctx.enter_context(tc.tile_pool(name="x", bufs=xbufs))
        itpool = ctx.enter_context(tc.tile_pool(name="it", bufs=itbufs))
        scr = ctx.enter_context(tc.tile_pool(name="scr", bufs=scrbufs))
        if lgbufs is None:
            lgbufs = group // 2 if pair_lg else group
        tp_ps = ctx.enter_context(tc.tile_pool(name="tp", bufs=tpbufs,
                                               space="PSUM"))
        gr_ps = ctx.enter_context(tc.tile_pool(name="gr", bufs=grbufs,
                                               space="PSUM"))
        lg_ps = ctx.enter_context(tc.tile_pool(name="lg", bufs=lgbufs,
                                               space="PSUM"))
        HD = DC // 2 * P  # 384: transpose-psum half width

        ident = consts.tile([P, P], f32)
        make_identity(nc, ident)
        ident_b = consts.tile([P, P], bf16)
        make_identity(nc, ident_b)

        zero_c = consts.tile([P, 1], f32)
        nc.vector.memset(zero_c, 0.0)
        ln2_c = consts.tile([P, 1], f32)
        nc.vector.memset(ln2_c, s_bias)

        mask_i = consts.tile([P, NT], dt.int32)
        nc.sync.dma_start(out=mask_i, in_=maskT)
        maskf = consts.tile([P, NT], f32)
        nc.gpsimd.tensor_copy(out=maskf, in_=mask_i)

        # ---- bag prep: bagnT[:, j*K:(j+1)*K] = (bag_n^T)[d-chunk j] ----
        bagnT = consts.tile([P, DC * K], fp8 if use_fp8 else bf16)
        for kc in range(2):
            bXf = scr.tile([P, D], f32, tag="sq")
            nc.sync.dma_start(out=bXf, in_=bag[kc * P:(kc + 1) * P, :])
            bscr = scr.tile([P, D], f32, tag="sq2")
            bss = consts.tile([P, 1], f32, tag=f"bss{kc}")
            nc.scalar.activation(out=bscr, in_=bXf, func=AF.Square,
                                 bias=zero_c, accum_out=bss)
            nc.vector.tensor_scalar_max(bss, bss, EPS2)
            nc.scalar.activation(out=bss, in_=bss, func=AF.Ln, bias=zero_c)
            nc.scalar.activation(out=bss, in_=bss, func=AF.Exp, scale=-0.5,
                                 bias=zero_c)
            bX = xpool.tile([P, D], bf16, tag="x")
            nc.vector.tensor_scalar(out=bX, in0=bXf, scalar1=bss,
                                    scalar2=None, op0=ALU.mult)
            tpb = tp_ps.tile([P, D], bf16, tag="tp")
            for j in range(DC):
                nc.tensor.transpose(tpb[:, j * P:(j + 1) * P],
                                    bX[:, j * P:(j + 1) * P], ident_b)
            for j in range(DC):
                dst = bagnT[:, j * K + kc * P: j * K + kc * P + P]
                if use_fp8:
                    nc.scalar.activation(out=dst,
                                         in_=tpb[:, j * P:(j + 1) * P],
                                         func=AF.Copy, scale=SC)
                else:
                    nc.scalar.copy(out=dst, in_=tpb[:, j * P:(j + 1) * P])

        ss_buf = consts.tile([P, NT], f32)
        sc1_buf = consts.tile([P, NT], f32)
        sc2_buf = consts.tile([P, NT], f32)
        s_buf = consts.tile([P, NT], f32)
        num_buf = consts.tile([P, NT], f32)
        den_buf = consts.tile([P, NT], f32)
        es_buf = consts.tile([P, NT], f32)

        xdt = fp8 if fp8_x else bf16
        inst3 = inst.rearrange("(t p) d -> t p d", p=P)
        x_tiles = {}

        def load_x(t):
            # SWDGE DMA with fp32 -> bf16 cast on the wire; `merge` row
            # tiles per DMA call to amortize descriptor generation.
            if t in x_tiles:
                return x_tiles.pop(t)
            xm = xpool.tile([P, merge, D], xdt, tag="x")
            nc.gpsimd.dma_start(
                out=xm, in_=inst3[t:t + merge, :, :].rearrange(
                    "t p d -> p t d"))
            for i in range(merge):
                x_tiles[t + i] = xm[:, i, :]
            return x_tiles.pop(t)

        for _rep in range(repeat):
            x_tiles.clear()
            for g in range(NT // group):
                lg_tiles = []
                lgp_tiles = {}
                for ti in range(group):
                    t = g * group + ti
                    X = load_x(t)
                    iT = itpool.tile([P, D], fp8 if use_fp8 else bf16,
                                     tag="it")
                    tpdt = xdt
                    if trans == "xbar":
                        for j in range(DC):
                            nc.sync.dma_start(
                                out=iT[:, j * P:(j + 1) * P],
                                in_=X[:, j * P:(j + 1) * P], transpose=True)
                    else:
                        tp = tp_ps.tile([P, D], tpdt, tag="tp")
                        for j in range(DC):
                            nc.tensor.transpose(tp[:, j * P:(j + 1) * P],
                                                X[:, j * P:(j + 1) * P],
                                                ident_b)
                        if cp_act > 0:
                            nc.scalar.copy(out=iT[:, :cp_act],
                                           in_=tp[:, :cp_act])
                        if cp_act < D:
                            nc.vector.tensor_copy(out=iT[:, cp_act:],
                                                  in_=tp[:, cp_act:])
                    gr = gr_ps.tile([P, P], f32, tag="gr")
                    if pair_lg:
                        if ti % 2 == 0:
                            lgp_tiles[ti // 2] = lg_ps.tile(
                                [P, 2 * K], f32, tag="lg", name="lgp")
                        lg = lgp_tiles[ti // 2][:, (ti % 2) * K:
                                                (ti % 2) * K + K]
                    else:
                        lg = lg_ps.tile([P, K], f32, tag="lg")
                    if use_fp8:
                        DR = mybir.MatmulPerfMode.DoubleRow
                        for jp in range(DC // 2):
                            blk2 = iT[:, 2 * jp * P:(2 * jp + 2) * P].rearrange(
                                "p (two c) -> p two c", two=2)
                            bg2 = bagnT[:, 2 * jp * K:(2 * jp + 2) * K].rearrange(
                                "p (two k) -> p two k", two=2)
                            nc.tensor.matmul(gr, lhsT=blk2, rhs=blk2,
                                             start=(jp == 0),
                                             stop=(jp == DC // 2 - 1),
                                             perf_mode=DR)
                            nc.tensor.matmul(lg, lhsT=blk2, rhs=bg2,
                                             start=(jp == 0),
                                             stop=(jp == DC // 2 - 1),
                                             perf_mode=DR)
                    else:
                        for j in range(DC):
                            blk = iT[:, j * P:(j + 1) * P]
                            nc.tensor.matmul(gr, lhsT=blk, rhs=blk,
                                             start=(j == 0),
                                             stop=(j == DC - 1))
                            nc.tensor.matmul(lg, lhsT=blk,
                                             rhs=bagnT[:, j * K:(j + 1) * K],
                                             start=(j == 0),
                                             stop=(j == DC - 1))
                    gscr = scr.tile([P, P], f32, tag="gscr")
                    nc.vector.tensor_mul(gscr, gr, ident)
                    nc.vector.reduce_sum(ss_buf[:, t:t + 1], gscr,
                                         axis=mybir.AxisListType.X)
                    lg_tiles.append(lg)

                gsl = slice(g * group, (g + 1) * group)
                # s = (1/T) * rsqrt(max(ss, eps^2)) = exp(-0.5*ln(ss') + ln2)
                nc.vector.tensor_scalar_max(sc1_buf[:, gsl],
                                            ss_buf[:, gsl], EPS2)
                nc.scalar.activation(out=sc2_buf[:, gsl],
                                     in_=sc1_buf[:, gsl],
                                     func=AF.Ln, bias=zero_c)
                nc.scalar.activation(out=s_buf[:, gsl], in_=sc2_buf[:, gsl],
                                     func=AF.Exp, scale=-0.5, bias=ln2_c)

                for ti in range(group):
                    t = g * group + ti
                    lg = lg_tiles[ti]
                    b_col = t // 2  # own-bag column (bag rolled per core)
                    s_col = s_buf[:, t:t + 1]
                    ex = scr.tile([P, K], f32, tag="ex")
                    nc.scalar.activation(out=ex, in_=lg[:, 0:K], func=AF.Exp,
                                         scale=s_col, bias=zero_c,
                                         accum_out=es_buf[:, t:t + 1])
                    nc.vector.tensor_tensor(out=num_buf[:, t:t + 1],
                                            in0=lg[:, b_col:b_col + 1],
                                            in1=s_col, op=ALU.mult)
                    nc.vector.tensor_sub(den_buf[:, t:t + 1],
                                         es_buf[:, t:t + 1],
                                         ex[:, b_col:b_col + 1])

        if dbg is not None:
            dbuf = consts.tile([P, 5 * NT], f32)
            for i, b in enumerate((ss_buf, sc1_buf, sc2_buf, s_buf, es_buf)):
                nc.vector.tensor_copy(out=dbuf[:, i * NT:(i + 1) * NT], in_=b)
            nc.sync.dma_start(out=dbg, in_=dbuf)
        ld = consts.tile([P, NT], f32)
        nc.scalar.activation(out=ld, in_=den_buf, func=AF.Ln, bias=zero_c)
        t1 = consts.tile([P, NT], f32)
        nc.vector.tensor_sub(t1, num_buf, ld)
        nc.vector.tensor_mul(t1, t1, maskf)
        outt = consts.tile([P, 2], f32)
        nc.vector.reduce_sum(outt[:, 0:1], t1, axis=mybir.AxisListType.X)
        nc.vector.reduce_sum(outt[:, 1:2], maskf, axis=mybir.AxisListType.X)
        nc.sync.dma_start(out=out, in_=outt)

    nc.compile()
    return nc


def _get(repeat=1, **kw):
    key = (repeat, tuple(sorted(kw.items())))
    if key not in _CACHE:
        _CACHE[key] = _build(repeat=repeat, **kw)
    return _CACHE[key]


def make_in_maps(instance_embedding, bag_embedding, mask):
    inst = np.ascontiguousarray(
        np.asarray(instance_embedding, dtype=np.float32).reshape(B * N, D))
    bagf = np.asarray(bag_embedding, dtype=np.float32)
    m = np.asarray(mask, dtype=np.int32).reshape(B * N)
    in_maps = []
    for c in range(NCORES):
        sh = inst[c * RPC:(c + 1) * RPC]
        bg = np.ascontiguousarray(np.roll(bagf, -c * BPC, axis=0))
        mt = np.ascontiguousarray(m[c * RPC:(c + 1) * RPC].reshape(NT, P).T)
        in_maps.append({"inst": sh, "bag": bg, "maskT": mt})
    return in_maps


def kernel(instance_embedding, bag_embedding, mask):
    from concourse import bass_utils

    nc = _get()
    in_maps = make_in_maps(instance_embedding, bag_embedding, mask)
    res = bass_utils.run_bass_kernel_spmd(nc, in_maps,
                                          core_ids=list(range(NCORES)))
    tsum = 0.0
    msum = 0.0
    for c in range(NCORES):
        o = res.results[c]["out"].astype(np.float64)
        tsum += o[:, 0].sum()
        msum += o[:, 1].sum()
    return np.array(-tsum / msum, dtype=np.float32)


if __name__ == "__main__":
    rng = np.random.default_rng(0)
    ie = rng.standard_normal((B, N, D), dtype=np.float32)
    be = rng.standard_normal((B, D), dtype=np.float32)
    mk = np.ones((B, N), dtype=np.int32)
    print("loss:", kernel(ie, be, mk))



# revision 30
# speedup vs baseline: 1647.8070x; 1647.8070x over previous
"""Trainium2 Bass kernel for the masked contrastive (MIL/NCE-style) loss.

Computes, for instance embeddings x[b,n,:], bag embeddings y[k,:]:
    logits[b,n,k] = cos(x[b,n], y[k]) / T
    loss = -mean_{mask}( logits[b,n,b] - logsumexp_{k!=b} logits[b,n,b] )

Strategy: data-parallel over bags across 8 NeuronCores (32 bags = 8192
instance rows per core). Every core receives the full bag embedding,
rolled by its bag offset so that each core's own-bag diagonal lands at a
fixed, compile-time-known column. Each core emits per-partition partial
sums of the masked per-instance terms and of the mask; the host does the
final (tiny) reduction and division.

Device-side layout: the host ships x as fp8e4m3 already transposed into
per-tile [d, r] layout (instT[t, d, j*128+r] = x[t*128+r, j*128+d]), so
the kernel needs no on-chip transposes or cast copies for x at all: each
row tile is 6 fp8 DoubleRow matmuls (3 Gram passes for the row norms +
3 logits passes against the resident bag matrix). Row norms come from
the Gram diagonal via one fused DVE multiply-reduce per tile; rsqrt is
exp(-0.5*ln(ss)) so ScalarE only ever needs the natural_log_exp_and_
others table set. exp(s*logits) runs per tile on ScalarE (per-partition
scale); its row sum comes from the activation accumulator for part of
the tiles and from DVE reductions (bf16, 2x mode) for the rest, to
balance the two engines. The own-bag numerator is recovered as
ln(exp(s*l_b)) from the exp scratch, so nothing ever reads single PSUM
columns. num/den bookkeeping is batched per quad of tiles on GPSIMD.

Dispatch: the jitted shard_map executable and the host-side
preprocessing jit are built once and cached; subsequent kernel() calls
only pay input transfer + execution (the generic per-call path re-jits
a fresh closure each call, re-shipping the NEFF-embedding HLO through
the axon tunnel every time).
"""

import math
import os
import sys

import numpy as np

for _p in ("/opt/trn_rl_repo",):
    if os.path.isdir(_p) and _p not in sys.path:
        sys.path.append(_p)

B, N, D = 256, 256, 768
NCORES = 8
BPC = B // NCORES          # bags per core = 32
RPC = BPC * N              # instance rows per core = 8192
P = 128                    # partitions
NT = RPC // P              # row tiles per core = 64
DC = D // P                # contraction chunks = 6
K = B                      # logits columns = 256
EPS2 = 1e-16               # eps^2 for the norm clamp (eps = 1e-8)
LN2 = 0.6931471805599453   # ln(2) == ln(1/T) for T=0.5
SC = 16.0                  # fp8 pre-scale on normalized bag rows

_CACHE = {}


def _patch_act_tables():
    """Prefer the natural_log_exp_and_others ACT table set so Exp, Ln,
    Square and Copy all resolve to ONE resident table (the default
    first-match order picks exp_and_others for Exp and natural_log for
    Ln, reloading tables repeatedly)."""
    import concourse.bacc as bacc
    import concourse.hw_specs as hw_specs

    if getattr(hw_specs, "_ct_patched", False):
        return
    orig = hw_specs.get_activation_tables

    def patched(module_arch):
        # Set order (and therefore act_func_set_id indices) must stay
        # identical to act_info.json, so instead of reordering we hide
        # Exp/Ln from every other set, forcing the chooser onto the
        # combined set.
        import concourse.mybir as mybir

        AF = mybir.ActivationFunctionType
        tabs = orig(module_arch)
        pref = "natural_log_exp_and_others"
        if pref not in tabs:
            return tabs
        return {
            name: (fns if name == pref else fns - {AF.Exp, AF.Ln})
            for name, fns in tabs.items()
        }

    hw_specs.get_activation_tables = patched
    hw_specs._ct_patched = True
    if getattr(bacc, "get_activation_tables", None) is orig:
        bacc.get_activation_tables = patched


def _build(repeat=1, merge=2, rg=8, acc8=1, xbufs=16, grbufs=2, lgbufs=6,
           exbufs=4, scrbufs=3):
    """Build + compile the single-core SPMD program.

    merge: row tiles loaded per DMA; rg: tiles per rsqrt batch; acc8: of
    every 4 tiles, how many use the ScalarE accumulator for the exp row
    sum (the rest get one batched DVE reduce per quad)."""
    from contextlib import ExitStack

    import concourse.bacc as bacc
    import concourse.mybir as mybir
    import concourse.tile as tile
    from concourse.masks import make_identity

    _patch_act_tables()

    dt = mybir.dt
    AF = mybir.ActivationFunctionType
    ALU = mybir.AluOpType
    f32 = dt.float32
    bf16 = dt.bfloat16
    fp8 = dt.float8e4
    s_bias = LN2 - math.log(SC)

    nc = bacc.Bacc("TRN2", target_bir_lowering=False, debug=False,
                   num_devices=NCORES)
    instT = nc.dram_tensor("instT", [NT, P, D], fp8,
                           kind="ExternalInput").ap()
    bag = nc.dram_tensor("bag", [K, D], f32, kind="ExternalInput").ap()
    maskT = nc.dram_tensor("maskT", [P, NT], dt.int32,
                           kind="ExternalInput").ap()
    out = nc.dram_tensor("out", [P, 2], f32, kind="ExternalOutput").ap()

    with tile.TileContext(nc) as tc, ExitStack() as ctx:
        consts = ctx.enter_context(tc.tile_pool(name="consts", bufs=1))
        xpool = ctx.enter_context(tc.tile_pool(name="x", bufs=xbufs))
        scr = ctx.enter_context(tc.tile_pool(name="scr", bufs=scrbufs))
        expool = ctx.enter_context(tc.tile_pool(name="ex", bufs=exbufs))
        gr_ps = ctx.enter_context(tc.tile_pool(name="gr", bufs=grbufs,
                                               space="PSUM"))
        lg_ps = ctx.enter_context(tc.tile_pool(name="lg", bufs=lgbufs,
                                               space="PSUM"))

        ident = consts.tile([P, P], f32)
        make_identity(nc, ident)
        ident_b = consts.tile([P, P], bf16)
        make_identity(nc, ident_b)
        ident4 = consts.tile([P, 4, P], f32)
        for i in range(4):
            nc.vector.tensor_copy(out=ident4[:, i, :], in_=ident)

        zero_c = consts.tile([P, 1], f32)
        nc.vector.memset(zero_c, 0.0)
        ln2_c = consts.tile([P, 1], f32)
        nc.vector.memset(ln2_c, s_bias)

        mask_i = consts.tile([P, NT], dt.int32)
        nc.sync.dma_start(out=mask_i, in_=maskT)
        maskf = consts.tile([P, NT], f32)
        nc.gpsimd.tensor_copy(out=maskf, in_=mask_i)

        # ---- bag prep: bagnT[:, j*K:(j+1)*K] = SC * (bag_n^T)[d-chunk j]
        # (emitted via a closure so the first x-tile group's loads/grams can
        # be queued ahead of it; bag DMAs ride the Activation HWDGE queue so
        # they don't sit behind the x loads on SP)
        bagnT = consts.tile([P, DC * K], fp8)

        bag_bX = {}

        def bag_norm(kc):
            # DMA + normalize one half of the bag rows (no PE work, so this
            # can be queued before the first x-tile group's grams)
            bXf = scr.tile([P, D], f32, tag="sq")
            nc.scalar.dma_start(out=bXf, in_=bag[kc * P:(kc + 1) * P, :])
            bscr = scr.tile([P, D], f32, tag="sq2")
            bss = consts.tile([P, 1], f32, tag=f"bss{kc}")
            # square+rowsum on DVE (AF.Square would drag in a second
            # activation table set; custom-DVE fused reduce ops crash the
            # exec unit on this runtime, so plain mul+reduce)
            nc.vector.tensor_mul(bscr, bXf, bXf)
            nc.vector.reduce_sum(bss, bscr, axis=mybir.AxisListType.X)
            nc.vector.tensor_scalar_max(bss, bss, EPS2)
            nc.scalar.activation(out=bss, in_=bss, func=AF.Ln, bias=zero_c)
            nc.scalar.activation(out=bss, in_=bss, func=AF.Exp, scale=-0.5,
                                 bias=zero_c)
            bX = scr.tile([P, D], bf16, tag=f"bx{kc}")
            nc.vector.tensor_scalar(out=bX, in0=bXf, scalar1=bss,
                                    scalar2=None, op0=ALU.mult)
            bag_bX[kc] = bX

        def bag_transp(kc):
            bX = bag_bX.pop(kc)
            tpb = lg_ps.tile([P, D], bf16, tag="lg", name="tpb")
            for j in range(DC):
                nc.tensor.transpose(tpb[:, j * P:(j + 1) * P],
                                    bX[:, j * P:(j + 1) * P], ident_b)
            # GPSIMD can't read PSUM on real HW; split PSUM->SBUF fp8
            # scale-copies between DVE and ScalarE (Copy shares the
            # resident table)
            for j in range(DC):
                dst = bagnT[:, j * K + kc * P: j * K + kc * P + P]
                if j % 2 == 0:
                    nc.vector.tensor_scalar_mul(dst,
                                                tpb[:, j * P:(j + 1) * P],
                                                SC)
                else:
                    nc.scalar.activation(out=dst,
                                         in_=tpb[:, j * P:(j + 1) * P],
                                         func=AF.Copy, scale=SC)

        ss_buf = consts.tile([P, NT], f32)
        s_buf = consts.tile([P, NT], f32)
        es_buf = consts.tile([P, NT], f32)
        exb_buf = consts.tile([P, NT], f32)
        den_buf = consts.tile([P, NT], f32)

        DR = mybir.MatmulPerfMode.DoubleRow
        AX = mybir.AxisListType.X
        instTr = instT.rearrange("t p d -> p t d")
        x_tiles = {}

        def load_x(t):
            # tiles stay in x_tiles until stage_b consumes them
            if t not in x_tiles:
                xm = xpool.tile([P, merge, D], fp8, tag="x")
                nc.sync.dma_start(out=xm, in_=instTr[:, t:t + merge, :])
                for i in range(merge):
                    x_tiles[t + i] = xm[:, i, :]
            return x_tiles[t]

        def pairs(ap2d):
            # [P, 768] -> per-pass DoubleRow views [P, 2, 128/256]
            return [ap2d[:, 2 * jp * P:(2 * jp + 2) * P].rearrange(
                "p (two c) -> p two c", two=2) for jp in range(DC // 2)]

        def bag_pairs():
            return [bagnT[:, 2 * jp * K:(2 * jp + 2) * K].rearrange(
                "p (two k) -> p two k", two=2) for jp in range(DC // 2)]

        bgp = None

        def stage_a(g):
            # loads + gram quads + diag extract + batched rsqrt for group g
            for q in range(g * rg // 4, (g + 1) * rg // 4):
                # 4 tiles' Gram matrices side by side in one PSUM bank so
                # one mul + one 3D reduce extracts all four diagonals
                grq = gr_ps.tile([P, 4, P], f32, tag="gr")
                for qi in range(4):
                    t = 4 * q + qi
                    xp = pairs(load_x(t))
                    for jp in range(DC // 2):
                        nc.tensor.matmul(grq[:, qi, :], lhsT=xp[jp],
                                         rhs=xp[jp], start=(jp == 0),
                                         stop=(jp == DC // 2 - 1),
                                         perf_mode=DR)
                gscr = scr.tile([P, 4, P], f32, tag="gscr")
                nc.vector.tensor_mul(gscr, grq, ident4)
                nc.vector.reduce_sum(ss_buf[:, 4 * q:4 * q + 4], gscr,
                                     axis=AX)
            gsl = slice(g * rg, (g + 1) * rg)
            # s = (1/T)/SC * rsqrt(max(ss, eps^2)) = exp(-0.5*ln(ss') + b)
            nc.vector.tensor_scalar_max(s_buf[:, gsl], ss_buf[:, gsl], EPS2)
            nc.scalar.activation(out=s_buf[:, gsl], in_=s_buf[:, gsl],
                                 func=AF.Ln, bias=zero_c)
            nc.scalar.activation(out=s_buf[:, gsl], in_=s_buf[:, gsl],
                                 func=AF.Exp, scale=-0.5, bias=ln2_c)

        def stage_b(g):
            # logits + exp + row sums + quad bookkeeping for tile group g
            ex_quads = {}
            for ti in range(rg):
                t = g * rg + ti
                X = x_tiles.pop(t)
                xp = pairs(X)
                lg = lg_ps.tile([P, K], f32, tag="lg")
                for jp in range(DC // 2):
                    nc.tensor.matmul(lg, lhsT=xp[jp], rhs=bgp[jp],
                                     start=(jp == 0),
                                     stop=(jp == DC // 2 - 1),
                                     perf_mode=DR)
                q, qi = t // 4, t % 4
                if qi == 0:
                    ex_quads[q] = expool.tile([P, 4, K], bf16, tag="exq",
                                              name="exq")
                exq = ex_quads[q]
                s_col = s_buf[:, t:t + 1]
                use_acc = qi < acc8
                nc.scalar.activation(
                    out=exq[:, qi, :], in_=lg, func=AF.Exp, scale=s_col,
                    bias=zero_c,
                    accum_out=(es_buf[:, t:t + 1] if use_acc else None))
                if not use_acc:
                    # [P,1] output keeps the DVE 2x (bf16-packed) mode
                    nc.vector.reduce_sum(es_buf[:, t:t + 1], exq[:, qi, :],
                                         axis=AX)
                if qi == 3:
                    # own-bag exp(s*l_b): tiles 4q..4q+3 have b_col
                    # 2q,2q,2q+1,2q+1 -> two [P,2,1] strided reads
                    b0 = 2 * q
                    nc.gpsimd.tensor_copy(
                        out=exb_buf[:, 4 * q:4 * q + 2],
                        in_=exq[:, 0:2, b0:b0 + 1])
                    nc.gpsimd.tensor_copy(
                        out=exb_buf[:, 4 * q + 2:4 * q + 4],
                        in_=exq[:, 2:4, b0 + 1:b0 + 2])
                    nc.gpsimd.tensor_sub(
                        den_buf[:, 4 * q:4 * q + 4],
                        es_buf[:, 4 * q:4 * q + 4],
                        exb_buf[:, 4 * q:4 * q + 4])

        num = consts.tile([P, NT], f32, tag="num")
        ld = consts.tile([P, NT], f32, tag="ld")
        t1 = consts.tile([P, NT], f32, tag="t1")

        def epilogue_half(h):
            # num = ln(exb) = s*l_b ; term = mask*(num - ln(den))
            hs = slice(h * NT // 2, (h + 1) * NT // 2)
            nc.scalar.activation(out=num[:, hs], in_=exb_buf[:, hs],
                                 func=AF.Ln, bias=zero_c)
            nc.scalar.activation(out=ld[:, hs], in_=den_buf[:, hs],
                                 func=AF.Ln, bias=zero_c)
            nc.vector.tensor_sub(t1[:, hs], num[:, hs], ld[:, hs])
            nc.vector.tensor_mul(t1[:, hs], t1[:, hs], maskf[:, hs])

        for _rep in range(repeat):
            x_tiles.clear()
            bgp = bag_pairs()
            ng = NT // rg
            if _rep == 0:
                bag_norm(0)
                bag_norm(1)
            stage_a(0)
            if _rep == 0:
                bag_transp(0)
                bag_transp(1)
            stage_a(1)
            for g in range(ng):
                if g + 2 < ng:
                    stage_a(g + 2)
                stage_b(g)
                if g == ng - 2:
                    epilogue_half(0)
            epilogue_half(1)
            outt = consts.tile([P, 2], f32, tag="outt")
            nc.vector.reduce_sum(outt[:, 0:1], t1, axis=AX)
            nc.vector.reduce_sum(outt[:, 1:2], maskf, axis=AX)
            nc.sync.dma_start(out=out, in_=outt)

    nc.compile()
    return nc


def _get(repeat=1, **kw):
    key = ("nc", repeat, tuple(sorted(kw.items())))
    if key not in _CACHE:
        _CACHE[key] = _build(repeat=repeat, **kw)
    return _CACHE[key]


# ---------------------------------------------------------------------------
# host-side preprocessing + cached dispatch
# ---------------------------------------------------------------------------

def _prep_fns():
    """jax-CPU jitted input marshalling (fp8 cast + per-tile transpose)."""
    if "prep" in _CACHE:
        return _CACHE["prep"]
    import jax
    import jax.numpy as jnp

    cpu = jax.devices("cpu")[0]

    def _prep_inst(x):
        # [B,N,D] -> cores x tiles x [d, j*128+r] fp8
        v = x.reshape(NCORES, NT, P, DC, P)      # [c, t, r, j, d]
        v = jnp.transpose(v, (0, 1, 4, 3, 2))    # [c, t, d, j, r]
        return v.astype(jnp.float8_e4m3fn).reshape(NCORES * NT, P, D)

    fn = jax.jit(_prep_inst, device=cpu)
    _CACHE["prep"] = fn
    return fn


def make_global_inputs(instance_embedding, bag_embedding, mask):
    inst = np.asarray(instance_embedding, dtype=np.float32)
    bagf = np.asarray(bag_embedding, dtype=np.float32)
    m = np.asarray(mask, dtype=np.int32)
    instT_g = np.asarray(_prep_fns()(inst))
    bag_g = np.concatenate(
        [np.roll(bagf, -c * BPC, axis=0) for c in range(NCORES)], axis=0)
    maskT_g = np.ascontiguousarray(
        m.reshape(NCORES, NT, P).transpose(0, 2, 1)).reshape(NCORES * P, NT)
    return {"instT": instT_g, "bag": bag_g, "maskT": maskT_g}


def _get_dispatch(repeat=1, **kw):
    """Build the shard_map executable for the compiled program ONCE.

    Mirrors concourse.bass2jax.run_bass_via_pjrt's multi-core branch but
    caches the jitted callable so warm calls skip re-tracing/re-lowering
    (the generic helper builds a fresh closure per call, which re-ships
    the NEFF-embedding HLO through the tunnel every time)."""
    key = ("disp", repeat, tuple(sorted(kw.items())))
    if key in _CACHE:
        return _CACHE[key]

    import jax
    from jax.sharding import Mesh, PartitionSpec
    from jax.experimental.shard_map import shard_map
    import concourse.mybir as mybir
    from concourse import bass2jax

    nc = _get(repeat=repeat, **kw)
    bass2jax.install_neuronx_cc_hook()
    assert nc.dbg_addr is None
    part_name = (nc.partition_id_tensor.name
                 if nc.partition_id_tensor is not None else None)

    in_names, out_names, out_avals, zero_shapes = [], [], [], []
    for alloc in nc.m.functions[0].allocations:
        if not isinstance(alloc, mybir.MemoryLocationSet):
            continue
        name = alloc.memorylocations[0].name
        if alloc.kind == "ExternalInput":
            if name != part_name:
                in_names.append(name)
        elif alloc.kind == "ExternalOutput":
            shape = tuple(alloc.tensor_shape)
            dtype = mybir.dt.np(alloc.dtype)
            out_names.append(name)
            out_avals.append(jax.core.ShapedArray(shape, dtype))
            zero_shapes.append((shape, dtype))
    n_params = len(in_names)
    all_names = in_names + out_names
    if part_name is not None:
        all_names = all_names + [part_name]
    donate = tuple(range(n_params, n_params + len(out_names)))

    def _body(*args):
        operands = list(args)
        if part_name is not None:
            operands.append(bass2jax.partition_id_tensor())
        outs = bass2jax._bass_exec_p.bind(
            *operands,
            out_avals=tuple(out_avals),
            in_names=tuple(all_names),
            out_names=tuple(out_names),
            lowering_input_output_aliases=(),
            sim_require_finite=True,
            sim_require_nnan=True,
            nc=nc,
        )
        return tuple(outs)

    devices = jax.devices()[:NCORES]
    mesh = Mesh(np.asarray(devices), ("core",))
    spec = PartitionSpec("core")
    sharded = jax.jit(
        shard_map(_body, mesh=mesh,
                  in_specs=(spec,) * (n_params + len(out_names)),
                  out_specs=(spec,) * len(out_names), check_rep=False),
        donate_argnums=donate, keep_unused=True)

    state = {
        "fn": sharded, "in_names": in_names, "out_names": out_names,
        "zero_shapes": zero_shapes, "mesh": mesh, "spec": spec,
    }
    _CACHE[key] = state
    return state


def _run(state, global_inputs):
    args = [global_inputs[n] for n in state["in_names"]]
    zeros = [np.zeros((NCORES * s[0], *s[1:]), d)
             for (s, d) in state["zero_shapes"]]
    outs = state["fn"](*args, *zeros)
    return {n: np.asarray(o) for n, o in zip(state["out_names"], outs)}


def kernel(instance_embedding, bag_embedding, mask):
    state = _get_dispatch(repeat=1)
    gin = make_global_inputs(instance_embedding, bag_embedding, mask)
    res = _run(state, gin)
    o = res["out"].astype(np.float64).reshape(NCORES, P, 2)
    tsum = o[:, :, 0].sum()
    msum = o[:, :, 1].sum()
    return np.array(-tsum / msum, dtype=np.float32)


if __name__ == "__main__":
    rng = np.random.default_rng(0)
    ie = rng.standard_normal((B, N, D), dtype=np.float32)
    be = rng.standard_normal((B, D), dtype=np.float32)
    mk = np.ones((B, N), dtype=np.int32)
    print("loss:", kernel(ie, be, mk))
